# revision 10
# baseline (speedup 1.0000x reference)
"""GCN layer (GCNConv + BatchNorm1d + ReLU + residual) on 8 Trainium2 cores.

Strategy (v4):
  - Nodes sharded 8 ways (6250/core); edges partitioned by destination core,
    grouped by (superblock, source-half, dest block), padded to 128-edge
    tiles. Self-loops are handled by a PE transpose-accumulate (no gather).
  - Host ships x' = x*dinv as fp16 plus packed int16 gather indices and a
    per-edge relative-dest-column stream. The remaining dinv[dst] factor is
    a per-column scale folded into the finalize PSUM->SBUF copy.
  - Blocking dma_gather of 1024 fp16 rows (256B each) per call, optionally
    round-robined over multiple SWDGE queues; selection-matrix matmuls in
    fp16 accumulate agg[feat,dst] in PSUM (S[e,d] = (colrel[e]==d), one DVE
    is_equal per tile).
  - Per dest block: fin = aggT @ W.T (fp16); h = fin * dinv[dst]; BN batch
    stats via ones-vector matmuls + one 8-core AllReduce of [2,128];
    out = x + relu(h*s + t) in fp16.
"""

import os
import sys

sys.path.insert(0, "/opt/trn_rl_repo")

import numpy as np

import concourse.bacc as bacc
import concourse.mybir as mybir
import concourse.tile as tile
from concourse.bass_utils import run_bass_kernel_spmd
from concourse.masks import make_identity

P = 128
D = 128
F32 = mybir.dt.float32
F16 = mybir.dt.float16
I32 = mybir.dt.int32
I16 = mybir.dt.int16
BN_EPS = 1e-5
CORES = 8
SBW = 5    # dest blocks per superblock (psum: 5 agg + fin + sh + sh2 = 8)
GCH = 1024  # max idxs per dma_gather call (HW limit)
NQ = int(os.environ.get("KNQ", "4"))  # SWDGE queues to round-robin


# ---------------------------------------------------------------- host prep
def _build_plan(x, edge_index, n_nodes):
    N = n_nodes
    npc = N // CORES
    nblk = (npc + P - 1) // P
    npad_local = nblk * P
    NPAD = ((N + P - 1) // P) * P
    GRP = ((NPAD // 2 + P - 1) // P) * P
    assert GRP <= 32767 and NPAD - GRP <= 32767

    src = np.asarray(edge_index[0]).astype(np.int64).astype(np.int32)
    dst = np.asarray(edge_index[1]).astype(np.int64).astype(np.int32)
    deg = (np.bincount(dst, minlength=N) + 1).astype(np.float32)
    dinv = 1.0 / np.sqrt(deg)
    # self-loops are NOT in the gather streams; they are added on-device via
    # a PE transpose of x'_own (their weight dinv[n]^2 = prescale * dinvb).

    # balance per-(core, block) edge counts with a node->slot permutation
    # (greedy: heaviest nodes first into the least-loaded bin with capacity).
    # Output rows come back slot-ordered; kernel() de-permutes on host.
    import heapq
    nbin = CORES * nblk
    cap = np.full(nbin, P, np.int64)
    for c in range(CORES):
        cap[c * nblk + nblk - 1] = npc - (nblk - 1) * P
    node_order = np.argsort(-deg, kind="stable")
    heap = [(0.0, float(b)) for b in range(nbin)]
    heapq.heapify(heap)
    fill = np.zeros(nbin, np.int64)
    perm = np.empty(N, np.int64)  # node -> global slot (core*npc + slot)
    ew = (deg - 1).astype(np.float64)  # in-edges per node (gathered work)
    for n in node_order:
        while True:
            s, bf = heapq.heappop(heap)
            b = int(bf)
            if fill[b] < cap[b]:
                break
        c, blk = divmod(b, nblk)
        perm[n] = c * npc + blk * P + fill[b]
        fill[b] += 1
        if fill[b] < cap[b]:
            heapq.heappush(heap, (s + ew[n], bf))
    assert (fill == cap).all()

    dslot = perm[dst].astype(np.int32)
    core_of = dslot // npc
    dloc = dslot - core_of * npc
    db_l = dloc // P
    g_l = (src >= GRP).astype(np.int32)
    sb_l = db_l // SBW
    nsb = (nblk + SBW - 1) // SBW

    order = np.lexsort((src, db_l, g_l, sb_l, core_of))
    src_s, dloc_s = src[order], dloc[order]
    core_s, db_s, g_s = core_of[order], db_l[order], g_l[order]

    cnt = np.zeros((CORES, nblk, 2), np.int64)
    np.add.at(cnt, (core_s, db_s, g_s), 1)
    T = ((cnt.max(axis=0) + P - 1) // P).astype(np.int64)  # [nblk, 2]

    offs = np.zeros((CORES, nsb, 2, SBW), np.int64)
    run = 0
    for c in range(CORES):
        for sb in range(nsb):
            for g in range(2):
                for j in range(SBW):
                    db = sb * SBW + j
                    if db >= nblk:
                        continue
                    offs[c, sb, g, j] = run
                    run += cnt[c, db, g]
    assert run == src.shape[0]

    tiles_total = int(T.sum())
    tot_e = tiles_total * P
    s_tot = tot_e // 16

    idx_streams = np.zeros((CORES, tot_e), np.int16)
    cr_streams = np.full((CORES, tot_e), -1.0, np.float32)
    n_idx_sb = np.zeros((nsb, 2), np.int64)
    seg_tile0 = np.zeros((nsb, 2), np.int64)
    pos = 0
    tcur = 0
    slot_pos = np.zeros((nsb, 2, SBW), np.int64)
    for sb in range(nsb):
        for g in range(2):
            seg_tile0[sb, g] = tcur
            for j in range(SBW):
                db = sb * SBW + j
                if db >= nblk:
                    continue
                slot_pos[sb, g, j] = pos
                w = int(T[db, g]) * P
                n_idx_sb[sb, g] += w
                pos += w
                tcur += int(T[db, g])
    assert pos == tot_e and tcur == tiles_total

    for c in range(CORES):
        for sb in range(nsb):
            for g in range(2):
                for j in range(SBW):
                    db = sb * SBW + j
                    if db >= nblk:
                        continue
                    k = int(cnt[c, db, g])
                    if k == 0:
                        continue
                    o = int(offs[c, sb, g, j])
                    p0 = int(slot_pos[sb, g, j])
                    idx_streams[c, p0 : p0 + k] = (
                        src_s[o : o + k] - g * GRP
                    ).astype(np.int16)
                    cr_streams[c, p0 : p0 + k] = (
                        dloc_s[o : o + k] - db * P
                    ).astype(np.float32)

    idx16 = np.zeros((CORES, P, s_tot), np.int16)
    colrel = np.zeros((CORES, P, tiles_total), np.float32)
    for c in range(CORES):
        idx16[c] = np.tile(idx_streams[c].reshape(-1, 16).T, (8, 1))
        colrel[c] = cr_streams[c].reshape(-1, P).T

    xp16 = np.zeros((NPAD, D), np.float16)
    xp16[:N] = x * dinv[:, None]
    inv_perm = np.empty(N, np.int64)  # global slot -> node
    inv_perm[perm] = np.arange(N)
    xp = x * dinv[:, None]
    xres16 = np.zeros((CORES, npad_local, D), np.float16)
    xo32 = np.zeros((CORES, npad_local, D), np.float32)  # x' own shard
    dinvb = np.ones((CORES, P, nblk), np.float32)
    for c in range(CORES):
        nodes_c = inv_perm[c * npc : (c + 1) * npc]
        xres16[c, :npc] = x[nodes_c]
        xo32[c, :npc] = xp[nodes_c]
        dv = np.ones(npad_local, np.float32)
        dv[:npc] = dinv[nodes_c]
        dinvb[c] = dv.reshape(nblk, P).T

    return dict(
        N=N, npc=npc, nblk=nblk, npad_local=npad_local, NPAD=NPAD, GRP=GRP,
        nsb=nsb, T=T, n_idx_sb=n_idx_sb, seg_tile0=seg_tile0,
        tiles_total=tiles_total, s_tot=s_tot, idx16=idx16, colrel=colrel,
        dinvb=dinvb, xp16=xp16, xres16=xres16, xo32=xo32, perm=perm,
    )


# ------------------------------------------------------------- device build
def _build_program(plan):
    N = plan["N"]
    nblk, nsb = plan["nblk"], plan["nsb"]
    NPAD, GRP = plan["NPAD"], plan["GRP"]
    npc, npad_local = plan["npc"], plan["npad_local"]
    T = plan["T"]
    n_idx_sb = plan["n_idx_sb"]
    seg_tile0 = plan["seg_tile0"]
    tiles_total, s_tot = plan["tiles_total"], plan["s_tot"]

    nc = bacc.Bacc("TRN2", target_bir_lowering=False, debug=False,
                   num_devices=CORES, num_swdge_queues=NQ)

    xp_d = nc.declare_dram_parameter("xp16", [NPAD, D], F16, isOutput=False)
    xres_d = nc.declare_dram_parameter("xres16", [npad_local, D], F16,
                                       isOutput=False)
    xo_d = nc.declare_dram_parameter("xo32", [npad_local, D], F32,
                                     isOutput=False)
    wt_d = nc.declare_dram_parameter("wt16", [D, D], F16, isOutput=False)
    gamma_d = nc.declare_dram_parameter("gamma", [1, D], F32, isOutput=False)
    beta_d = nc.declare_dram_parameter("beta", [1, D], F32, isOutput=False)
    idx_d = nc.declare_dram_parameter("idx16", [P, s_tot], I16, isOutput=False)
    cr_d = nc.declare_dram_parameter("colrel", [P, tiles_total], F32,
                                     isOutput=False)
    dinvb_d = nc.declare_dram_parameter("dinvb", [P, nblk], F32,
                                        isOutput=False)
    out_d = nc.declare_dram_parameter("out", [npc, D], F16, isOutput=True)

    cc_in = nc.dram_tensor("cc_in", [2, D], F32)
    cc_out = nc.dram_tensor("cc_out", [2, D], F32, addr_space="Shared")

    with tile.TileContext(nc) as tc:
        with tc.tile_pool(name="const", bufs=1) as cpool, \
             tc.tile_pool(name="work", bufs=4) as wpool, \
             tc.tile_pool(name="gath", bufs=6) as gpool, \
             tc.tile_pool(name="psum", bufs=1, space="PSUM") as ppool:

            # ---- constants
            KMAX = int(T.max())
            iota_i = cpool.tile([P, KMAX * P], I32)
            nc.gpsimd.iota(iota_i[:], pattern=[[0, KMAX], [1, P]], base=0,
                           channel_multiplier=0)
            iota_h = cpool.tile([P, KMAX * P], F16)
            nc.vector.tensor_copy(iota_h[:], iota_i[:])

            ident = cpool.tile([P, P], F32)
            make_identity(nc, ident[:])

            wt_sb = cpool.tile([D, D], F16)
            nc.sync.dma_start(out=wt_sb[:], in_=wt_d[:, :])
            gamma_sb = cpool.tile([1, D], F32)
            nc.sync.dma_start(out=gamma_sb[:], in_=gamma_d[:, :])
            beta_sb = cpool.tile([1, D], F32)
            nc.sync.dma_start(out=beta_sb[:], in_=beta_d[:, :])

            ones16 = cpool.tile([P, 1], F16)
            nc.vector.memset(ones16[:], 1.0)
            ones_row = cpool.tile([1, P], F32)
            nc.vector.memset(ones_row[:], 1.0)

            idx_sb = cpool.tile([P, s_tot], I16)
            nc.sync.dma_start(out=idx_sb[:], in_=idx_d[:, :])
            cr_sb = cpool.tile([P, tiles_total], F32)
            nc.sync.dma_start(out=cr_sb[:], in_=cr_d[:, :])
            dinvb_sb = cpool.tile([P, nblk], F32)
            nc.sync.dma_start(out=dinvb_sb[:], in_=dinvb_d[:, :])

            xres_sb = cpool.tile([P, nblk * P], F16)
            nc.sync.dma_start(
                out=xres_sb[:].rearrange("p (a k) -> p a k", k=P),
                in_=xres_d[:, :].rearrange("(a p) k -> p a k", p=P))
            xo_sb = cpool.tile([P, nblk * P], F32)
            nc.sync.dma_start(
                out=xo_sb[:].rearrange("p (a k) -> p a k", k=P),
                in_=xo_d[:, :].rearrange("(a p) k -> p a k", p=P))

            h_buf = cpool.tile([P, nblk * P], F16)

            sh_ps = ppool.tile([1, D], F32, tag="sh", name="sh_ps")
            sh2_ps = ppool.tile([1, D], F32, tag="sh2", name="sh2_ps")

            qctr = [0]

            def emit_gather(sb, g):
                n_idx = int(n_idx_sb[sb, g])
                if n_idx == 0:
                    return None
                gt = gpool.tile([P, n_idx], F16, tag="gt",
                                name=f"gt_{sb}_{g}")
                c0 = int(seg_tile0[sb, g]) * (P // 16)
                for k0 in range(0, n_idx, GCH):
                    n = min(GCH, n_idx - k0)
                    q = qctr[0] % NQ
                    qctr[0] += 1
                    nc.gpsimd.dma_gather(
                        out_ap=gt[:, k0 : k0 + n].rearrange(
                            "p (t k) -> p t k", k=P),
                        in_ap=xp_d[g * GRP : min((g + 1) * GRP, NPAD), :],
                        idxs_ap=idx_sb[:, c0 + k0 // 16 : c0 + (k0 + n) // 16],
                        num_idxs=n, num_idxs_reg=n, elem_size=D,
                        queue_num=q)
                return gt

            # ---- main pass
            for sb in range(nsb):
                blks = list(range(sb * SBW, min((sb + 1) * SBW, nblk)))
                gts = [emit_gather(sb, 0), emit_gather(sb, 1)]
                loc0 = {0: 0, 1: 0}
                locs = {}
                for db in blks:
                    for g in range(2):
                        locs[(db, g)] = loc0[g]
                        loc0[g] += int(T[db, g])
                for j, db in enumerate(blks):
                    ntot = int(T[db, 0]) + int(T[db, 1])
                    psum = ppool.tile([P, P], F32, tag=f"agg{j}",
                                      name=f"agg_{db}")
                    # self-loop: agg += transpose(x'_own block)
                    nc.tensor.matmul(
                        out=psum[:], lhsT=xo_sb[:, db * P : (db + 1) * P],
                        rhs=ident[:], is_transpose=True,
                        start=True, stop=(ntot == 0))
                    k = 0
                    for g in range(2):
                        gt = gts[g]
                        Tg = int(T[db, g])
                        if Tg == 0:
                            continue
                        t0 = int(seg_tile0[sb, g]) + locs[(db, g)]
                        s_w = wpool.tile([P, Tg * P], F16, tag="s_t",
                                         name=f"s_{t0}")
                        nc.vector.tensor_tensor(
                            out=s_w[:], in0=iota_h[:, : Tg * P],
                            in1=cr_sb[:, t0 : t0 + Tg].to_broadcast(
                                [P, Tg, P]),
                            op=mybir.AluOpType.is_equal)
                        for t in range(Tg):
                            loc = locs[(db, g)] + t
                            nc.tensor.matmul(
                                out=psum[:],
                                lhsT=gt[:, loc * P : (loc + 1) * P],
                                rhs=s_w[:, t * P : (t + 1) * P],
                                start=False, stop=(k == ntot - 1))
                            k += 1
                    # finalize: fin = aggT @ W.T; h = fin * dinv[dst]
                    aggt = wpool.tile([P, P], F16, tag="aggt",
                                      name=f"aggt_{db}")
                    nc.scalar.activation(aggt[:], psum[:],
                                         mybir.ActivationFunctionType.Copy)
                    fin = ppool.tile([P, P], F32, tag="fin",
                                     name=f"fin_{db}")
                    nc.tensor.matmul(out=fin[:], lhsT=aggt[:], rhs=wt_sb[:],
                                     start=True, stop=True)
                    hb = h_buf[:, db * P : (db + 1) * P]
                    nc.scalar.activation(hb, fin[:],
                                         mybir.ActivationFunctionType.Copy,
                                         scale=dinvb_sb[:, db : db + 1])
                    sq = wpool.tile([P, P], F16, tag="sq",
                                    name=f"sq_{db}")
                    nc.scalar.activation(sq[:], hb,
                                         mybir.ActivationFunctionType.Square)
                    nc.tensor.matmul(out=sh_ps[:], lhsT=ones16[:],
                                     rhs=hb, start=(db == 0),
                                     stop=(db == nblk - 1))
                    nc.tensor.matmul(out=sh2_ps[:], lhsT=ones16[:],
                                     rhs=sq[:], start=(db == 0),
                                     stop=(db == nblk - 1))

            # ---- BN stats reduce + normalize constants
            _no_cc = os.environ.get("KNO_CC", "") == "1"
            sh_sb = cpool.tile([1, D], F32)
            nc.vector.tensor_copy(sh_sb[:], sh_ps[:])
            sh2_sb = cpool.tile([1, D], F32)
            nc.vector.tensor_copy(sh2_sb[:], sh2_ps[:])
            gsum = cpool.tile([1, D], F32)
            gsum2 = cpool.tile([1, D], F32)
            if _no_cc:
                nc.vector.tensor_scalar_mul(gsum[:], sh_sb[:], float(CORES))
                nc.vector.tensor_scalar_mul(gsum2[:], sh2_sb[:], float(CORES))
            else:
                nc.sync.dma_start(out=cc_in[0:1, :], in_=sh_sb[:])
                nc.sync.dma_start(out=cc_in[1:2, :], in_=sh2_sb[:])
                nc.gpsimd.collective_compute(
                    "AllReduce", mybir.AluOpType.add,
                    ins=[cc_in[:]], outs=[cc_out[:]],
                    replica_groups=[list(range(CORES))])
                nc.sync.dma_start(out=gsum[:], in_=cc_out[0:1, :])
                nc.sync.dma_start(out=gsum2[:], in_=cc_out[1:2, :])

            mean = cpool.tile([1, D], F32)
            nc.vector.tensor_scalar_mul(mean[:], gsum[:], 1.0 / N)
            eh2 = cpool.tile([1, D], F32)
            nc.vector.tensor_scalar_mul(eh2[:], gsum2[:], 1.0 / N)
            msq = cpool.tile([1, D], F32)
            nc.vector.tensor_tensor(out=msq[:], in0=mean[:], in1=mean[:],
                                    op=mybir.AluOpType.mult)
            var = cpool.tile([1, D], F32)
            nc.vector.tensor_tensor(out=var[:], in0=eh2[:], in1=msq[:],
                                    op=mybir.AluOpType.subtract)
            vare = cpool.tile([1, D], F32)
            nc.vector.tensor_scalar_add(vare[:], var[:], BN_EPS)
            sdev = cpool.tile([1, D], F32)
            nc.scalar.activation(sdev[:], vare[:],
                                 mybir.ActivationFunctionType.Sqrt)
            rstd = cpool.tile([1, D], F32)
            nc.vector.reciprocal(rstd[:], sdev[:])

            st_row = cpool.tile([1, 2 * D], F32)
            nc.vector.tensor_tensor(out=st_row[:, :D], in0=rstd[:],
                                    in1=gamma_sb[:], op=mybir.AluOpType.mult)
            ms = cpool.tile([1, D], F32)
            nc.vector.tensor_tensor(out=ms[:], in0=mean[:],
                                    in1=st_row[:, :D],
                                    op=mybir.AluOpType.mult)
            nc.vector.tensor_tensor(out=st_row[:, D:], in0=beta_sb[:],
                                    in1=ms[:], op=mybir.AluOpType.subtract)
            bc_ps = ppool.tile([P, 2 * D], F32, tag="fin")
            nc.tensor.matmul(out=bc_ps[:], lhsT=ones_row[:], rhs=st_row[:],
                             start=True, stop=True)
            s_rep = cpool.tile([P, D], F16)
            nc.vector.tensor_copy(s_rep[:], bc_ps[:, :D])
            t_rep = cpool.tile([P, D], F16)
            nc.vector.tensor_copy(t_rep[:], bc_ps[:, D:])
            OW = 1024
            s_w = cpool.tile([P, OW], F16)
            t_w = cpool.tile([P, OW], F16)
            for a in range(OW // P):
                nc.vector.tensor_copy(s_w[:, a * P : (a + 1) * P], s_rep[:])
                nc.vector.tensor_copy(t_w[:, a * P : (a + 1) * P], t_rep[:])

            # ---- phase 3: out = xres + relu(h*s + t), 1024-wide chunks
            nb3 = (npad_local + OW - 1) // OW
            for b in range(nb3):
                w = min(OW, npad_local - b * OW)
                ot = wpool.tile([P, w], F16, tag="ot")
                hslice = h_buf[:, b * OW : b * OW + w]
                tmp = wpool.tile([P, w], F16, tag="p3", name=f"p3_{b}")
                nc.vector.tensor_tensor(out=tmp[:], in0=hslice,
                                        in1=s_w[:, :w],
                                        op=mybir.AluOpType.mult)
                nc.vector.tensor_tensor(out=tmp[:], in0=tmp[:],
                                        in1=t_w[:, :w],
                                        op=mybir.AluOpType.add)
                nc.scalar.activation(ot[:], tmp[:],
                                     mybir.ActivationFunctionType.Relu)
                nc.vector.tensor_tensor(
                    out=ot[:], in0=ot[:],
                    in1=xres_sb[:, b * OW : b * OW + w],
                    op=mybir.AluOpType.add)
                lo = b * OW
                hi = min(npc, lo + w)
                if hi <= lo:
                    continue
                nv = hi - lo
                full = nv // P
                if full > 0:
                    nc.sync.dma_start(
                        out=out_d[lo : lo + full * P, :].rearrange(
                            "(a p) k -> p a k", p=P),
                        in_=ot[:, : full * P].rearrange(
                            "p (a k) -> p a k", k=P))
                rem = nv - full * P
                if rem > 0:
                    nc.sync.dma_start(
                        out=out_d[lo + full * P : hi, :],
                        in_=ot[:rem, full * P : (full + 1) * P])

    nc.compile()
    return nc


# ------------------------------------------------------------------ driver
_CACHE = {}
TRACE = False
RUN_KWARGS = None
LAST_RESULT = None


def kernel(**inputs):
    x = np.asarray(inputs["x"], np.float32)
    edge_index = np.asarray(inputs["edge_index"])
    W = np.asarray(inputs["W"], np.float32)
    gamma = np.asarray(inputs["gamma"], np.float32)
    beta = np.asarray(inputs["beta"], np.float32)
    # inputs["b"] shifts h uniformly and cancels under batch-norm mean
    # subtraction, so it does not affect the output.
    N = x.shape[0]

    plan = _build_plan(x, edge_index, N)
    key = (N, edge_index.shape[1], plan["tiles_total"],
           tuple(plan["T"].ravel().tolist()))
    if key not in _CACHE:
        _CACHE[key] = _build_program(plan)
    nc = _CACHE[key]

    wt16 = np.ascontiguousarray(W.T).astype(np.float16)
    in_maps = []
    for c in range(CORES):
        in_maps.append({
            "xp16": plan["xp16"],
            "xres16": plan["xres16"][c],
            "xo32": plan["xo32"][c],
            "wt16": wt16,
            "gamma": gamma.reshape(1, -1),
            "beta": beta.reshape(1, -1),
            "idx16": plan["idx16"][c],
            "colrel": plan["colrel"][c],
            "dinvb": plan["dinvb"][c],
        })

    res = run_bass_kernel_spmd(nc, in_maps, list(range(CORES)),
                               trace=TRACE, **(RUN_KWARGS or {}))
    global LAST_RESULT
    LAST_RESULT = res
    out = np.concatenate([res.results[c]["out"] for c in range(CORES)],
                         axis=0)
    return out[plan["perm"]].astype(np.float32)


# revision 11
# speedup vs baseline: 1.5107x; 1.5107x over previous
"""GCN layer (GCNConv + BatchNorm1d + ReLU + residual) on 8 Trainium2 cores.

Strategy (v4):
  - Nodes sharded 8 ways (6250/core); edges partitioned by destination core,
    grouped by (superblock, source-half, dest block), padded to 128-edge
    tiles. Self-loops are handled by a PE transpose-accumulate (no gather).
  - Host ships x' = x*dinv as fp16 plus packed int16 gather indices and a
    per-edge relative-dest-column stream. The remaining dinv[dst] factor is
    a per-column scale folded into the finalize PSUM->SBUF copy.
  - Blocking dma_gather of 1024 fp16 rows (256B each) per call, optionally
    round-robined over multiple SWDGE queues; selection-matrix matmuls in
    fp16 accumulate agg[feat,dst] in PSUM (S[e,d] = (colrel[e]==d), one DVE
    is_equal per tile).
  - Per dest block: fin = aggT @ W.T (fp16); h = fin * dinv[dst]; BN batch
    stats via ones-vector matmuls + one 8-core AllReduce of [2,128];
    out = x + relu(h*s + t) in fp16.
"""

import os
import sys

sys.path.insert(0, "/opt/trn_rl_repo")

import numpy as np

import concourse.bacc as bacc
import concourse.mybir as mybir
import concourse.tile as tile
from concourse.bass_utils import run_bass_kernel_spmd
from concourse.masks import make_identity

P = 128
D = 128
F32 = mybir.dt.float32
F16 = mybir.dt.float16
I32 = mybir.dt.int32
I16 = mybir.dt.int16
BN_EPS = 1e-5
CORES = 8
SBW = 5    # dest blocks per superblock (psum: 5 agg + fin + sh + sh2 = 8)
GCH = 1024  # max idxs per dma_gather call (HW limit)
NQ = int(os.environ.get("KNQ", "4"))  # SWDGE queues to round-robin


# ---------------------------------------------------------------- host prep
def _build_plan(x, edge_index, n_nodes):
    N = n_nodes
    npc = N // CORES
    nblk = (npc + P - 1) // P
    npad_local = nblk * P
    NPAD = ((N + P - 1) // P) * P
    GRP = ((NPAD // 2 + P - 1) // P) * P
    assert GRP <= 32767 and NPAD - GRP <= 32767

    src = np.asarray(edge_index[0]).astype(np.int64).astype(np.int32)
    dst = np.asarray(edge_index[1]).astype(np.int64).astype(np.int32)
    deg = (np.bincount(dst, minlength=N) + 1).astype(np.float32)
    dinv = 1.0 / np.sqrt(deg)
    # self-loops are NOT in the gather streams; they are added on-device via
    # a PE transpose of x'_own (their weight dinv[n]^2 = prescale * dinvb).

    # balance per-(core, block) edge counts with a node->slot permutation
    # (greedy: heaviest nodes first into the least-loaded bin with capacity).
    # Output rows come back slot-ordered; kernel() de-permutes on host.
    import heapq
    nbin = CORES * nblk
    cap = np.full(nbin, P, np.int64)
    for c in range(CORES):
        cap[c * nblk + nblk - 1] = npc - (nblk - 1) * P
    node_order = np.argsort(-deg, kind="stable")
    heap = [(0.0, float(b)) for b in range(nbin)]
    heapq.heapify(heap)
    fill = np.zeros(nbin, np.int64)
    perm = np.empty(N, np.int64)  # node -> global slot (core*npc + slot)
    ew = (deg - 1).astype(np.float64)  # in-edges per node (gathered work)
    for n in node_order:
        while True:
            s, bf = heapq.heappop(heap)
            b = int(bf)
            if fill[b] < cap[b]:
                break
        c, blk = divmod(b, nblk)
        perm[n] = c * npc + blk * P + fill[b]
        fill[b] += 1
        if fill[b] < cap[b]:
            heapq.heappush(heap, (s + ew[n], bf))
    assert (fill == cap).all()

    dslot = perm[dst].astype(np.int32)
    core_of = dslot // npc
    dloc = dslot - core_of * npc
    db_l = dloc // P
    g_l = (src >= GRP).astype(np.int32)
    sb_l = db_l // SBW
    nsb = (nblk + SBW - 1) // SBW

    order = np.lexsort((src, db_l, g_l, sb_l, core_of))
    src_s, dloc_s = src[order], dloc[order]
    core_s, db_s, g_s = core_of[order], db_l[order], g_l[order]

    cnt = np.zeros((CORES, nblk, 2), np.int64)
    np.add.at(cnt, (core_s, db_s, g_s), 1)
    T = ((cnt.max(axis=0) + P - 1) // P).astype(np.int64)  # [nblk, 2]

    offs = np.zeros((CORES, nsb, 2, SBW), np.int64)
    run = 0
    for c in range(CORES):
        for sb in range(nsb):
            for g in range(2):
                for j in range(SBW):
                    db = sb * SBW + j
                    if db >= nblk:
                        continue
                    offs[c, sb, g, j] = run
                    run += cnt[c, db, g]
    assert run == src.shape[0]

    tiles_total = int(T.sum())
    tot_e = tiles_total * P
    s_tot = tot_e // 16

    idx_streams = np.zeros((CORES, tot_e), np.int16)
    cr_streams = np.full((CORES, tot_e), -1.0, np.float32)
    n_idx_sb = np.zeros((nsb, 2), np.int64)
    seg_tile0 = np.zeros((nsb, 2), np.int64)
    pos = 0
    tcur = 0
    slot_pos = np.zeros((nsb, 2, SBW), np.int64)
    for sb in range(nsb):
        for g in range(2):
            seg_tile0[sb, g] = tcur
            for j in range(SBW):
                db = sb * SBW + j
                if db >= nblk:
                    continue
                slot_pos[sb, g, j] = pos
                w = int(T[db, g]) * P
                n_idx_sb[sb, g] += w
                pos += w
                tcur += int(T[db, g])
    assert pos == tot_e and tcur == tiles_total

    for c in range(CORES):
        for sb in range(nsb):
            for g in range(2):
                for j in range(SBW):
                    db = sb * SBW + j
                    if db >= nblk:
                        continue
                    k = int(cnt[c, db, g])
                    if k == 0:
                        continue
                    o = int(offs[c, sb, g, j])
                    p0 = int(slot_pos[sb, g, j])
                    idx_streams[c, p0 : p0 + k] = (
                        src_s[o : o + k] - g * GRP
                    ).astype(np.int16)
                    cr_streams[c, p0 : p0 + k] = (
                        dloc_s[o : o + k] - db * P
                    ).astype(np.float32)

    idx16 = np.zeros((CORES, P, s_tot), np.int16)
    colrel = np.zeros((CORES, P, tiles_total), np.float32)
    for c in range(CORES):
        idx16[c] = np.tile(idx_streams[c].reshape(-1, 16).T, (8, 1))
        colrel[c] = cr_streams[c].reshape(-1, P).T

    xp16 = np.zeros((NPAD, D), np.float16)
    xp16[:N] = x * dinv[:, None]
    inv_perm = np.empty(N, np.int64)  # global slot -> node
    inv_perm[perm] = np.arange(N)
    xp = x * dinv[:, None]
    xres16 = np.zeros((CORES, npad_local, D), np.float16)
    xo32 = np.zeros((CORES, npad_local, D), np.float32)  # x' own shard
    dinvb = np.ones((CORES, P, nblk), np.float32)
    for c in range(CORES):
        nodes_c = inv_perm[c * npc : (c + 1) * npc]
        xres16[c, :npc] = x[nodes_c]
        xo32[c, :npc] = xp[nodes_c]
        dv = np.ones(npad_local, np.float32)
        dv[:npc] = dinv[nodes_c]
        dinvb[c] = dv.reshape(nblk, P).T

    return dict(
        N=N, npc=npc, nblk=nblk, npad_local=npad_local, NPAD=NPAD, GRP=GRP,
        nsb=nsb, T=T, n_idx_sb=n_idx_sb, seg_tile0=seg_tile0,
        tiles_total=tiles_total, s_tot=s_tot, idx16=idx16, colrel=colrel,
        dinvb=dinvb, xp16=xp16, xres16=xres16, xo32=xo32, perm=perm,
    )


# ------------------------------------------------------------- device build
def _build_program(plan):
    N = plan["N"]
    nblk, nsb = plan["nblk"], plan["nsb"]
    NPAD, GRP = plan["NPAD"], plan["GRP"]
    npc, npad_local = plan["npc"], plan["npad_local"]
    T = plan["T"]
    n_idx_sb = plan["n_idx_sb"]
    seg_tile0 = plan["seg_tile0"]
    tiles_total, s_tot = plan["tiles_total"], plan["s_tot"]

    nc = bacc.Bacc("TRN2", target_bir_lowering=False, debug=False,
                   num_devices=CORES, num_swdge_queues=NQ)

    xp_d = nc.declare_dram_parameter("xp16", [NPAD, D], F16, isOutput=False)
    xres_d = nc.declare_dram_parameter("xres16", [npad_local, D], F16,
                                       isOutput=False)
    xo_d = nc.declare_dram_parameter("xo32", [npad_local, D], F32,
                                     isOutput=False)
    wt_d = nc.declare_dram_parameter("wt16", [D, D], F16, isOutput=False)
    gamma_d = nc.declare_dram_parameter("gamma", [1, D], F32, isOutput=False)
    beta_d = nc.declare_dram_parameter("beta", [1, D], F32, isOutput=False)
    idx_d = nc.declare_dram_parameter("idx16", [P, s_tot], I16, isOutput=False)
    cr_d = nc.declare_dram_parameter("colrel", [P, tiles_total], F32,
                                     isOutput=False)
    dinvb_d = nc.declare_dram_parameter("dinvb", [P, nblk], F32,
                                        isOutput=False)
    out_d = nc.declare_dram_parameter("out", [npc, D], F16, isOutput=True)

    cc_in = nc.dram_tensor("cc_in", [2, D], F32)
    cc_out = nc.dram_tensor("cc_out", [2, D], F32, addr_space="Shared")

    with tile.TileContext(nc) as tc:
        with tc.tile_pool(name="const", bufs=1) as cpool, \
             tc.tile_pool(name="work", bufs=4) as wpool, \
             tc.tile_pool(name="gath", bufs=4) as gpool, \
             tc.tile_pool(name="psum", bufs=1, space="PSUM") as ppool:

            # ---- constants
            KMAX = int(T.max())
            iota_i = cpool.tile([P, KMAX * P], I32)
            nc.gpsimd.iota(iota_i[:], pattern=[[0, KMAX], [1, P]], base=0,
                           channel_multiplier=0)
            iota_h = cpool.tile([P, KMAX * P], F16)
            nc.vector.tensor_copy(iota_h[:], iota_i[:])

            ident = cpool.tile([P, P], F32)
            make_identity(nc, ident[:])

            wt_sb = cpool.tile([D, D], F16)
            nc.sync.dma_start(out=wt_sb[:], in_=wt_d[:, :])
            gamma_sb = cpool.tile([1, D], F32)
            nc.sync.dma_start(out=gamma_sb[:], in_=gamma_d[:, :])
            beta_sb = cpool.tile([1, D], F32)
            nc.sync.dma_start(out=beta_sb[:], in_=beta_d[:, :])

            ones16 = cpool.tile([P, 1], F16)
            nc.vector.memset(ones16[:], 1.0)
            ones_row = cpool.tile([1, P], F32)
            nc.vector.memset(ones_row[:], 1.0)

            idx_sb = cpool.tile([P, s_tot], I16)
            nc.sync.dma_start(out=idx_sb[:], in_=idx_d[:, :])
            cr_sb = cpool.tile([P, tiles_total], F32)
            nc.sync.dma_start(out=cr_sb[:], in_=cr_d[:, :])
            dinvb_sb = cpool.tile([P, nblk], F32)
            nc.sync.dma_start(out=dinvb_sb[:], in_=dinvb_d[:, :])

            xres_sb = cpool.tile([P, nblk * P], F16)
            nc.sync.dma_start(
                out=xres_sb[:].rearrange("p (a k) -> p a k", k=P),
                in_=xres_d[:, :].rearrange("(a p) k -> p a k", p=P))
            xo_sb = cpool.tile([P, nblk * P], F32)
            nc.sync.dma_start(
                out=xo_sb[:].rearrange("p (a k) -> p a k", k=P),
                in_=xo_d[:, :].rearrange("(a p) k -> p a k", p=P))

            h_buf = cpool.tile([P, nblk * P], F16)

            sh_ps = ppool.tile([1, D], F32, tag="sh", name="sh_ps")
            sh2_ps = ppool.tile([1, D], F32, tag="sh2", name="sh2_ps")

            qctr = [0]

            def emit_gather(sb, g):
                n_idx = int(n_idx_sb[sb, g])
                if n_idx == 0:
                    return None
                gt = gpool.tile([P, n_idx], F16, tag="gt",
                                name=f"gt_{sb}_{g}")
                c0 = int(seg_tile0[sb, g]) * (P // 16)
                for k0 in range(0, n_idx, GCH):
                    n = min(GCH, n_idx - k0)
                    q = qctr[0] % NQ
                    qctr[0] += 1
                    nc.gpsimd.dma_gather(
                        out_ap=gt[:, k0 : k0 + n].rearrange(
                            "p (t k) -> p t k", k=P),
                        in_ap=xp_d[g * GRP : min((g + 1) * GRP, NPAD), :],
                        idxs_ap=idx_sb[:, c0 + k0 // 16 : c0 + (k0 + n) // 16],
                        num_idxs=n, num_idxs_reg=n, elem_size=D,
                        queue_num=q)
                return gt

            # ---- main pass
            for sb in range(nsb):
                blks = list(range(sb * SBW, min((sb + 1) * SBW, nblk)))
                gts = [emit_gather(sb, 0), emit_gather(sb, 1)]
                loc0 = {0: 0, 1: 0}
                locs = {}
                for db in blks:
                    for g in range(2):
                        locs[(db, g)] = loc0[g]
                        loc0[g] += int(T[db, g])
                for j, db in enumerate(blks):
                    ntot = int(T[db, 0]) + int(T[db, 1])
                    psum = ppool.tile([P, P], F32, tag=f"agg{j}",
                                      name=f"agg_{db}")
                    # self-loop: agg += transpose(x'_own block)
                    nc.tensor.matmul(
                        out=psum[:], lhsT=xo_sb[:, db * P : (db + 1) * P],
                        rhs=ident[:], is_transpose=True,
                        start=True, stop=(ntot == 0))
                    k = 0
                    for g in range(2):
                        gt = gts[g]
                        Tg = int(T[db, g])
                        if Tg == 0:
                            continue
                        t0 = int(seg_tile0[sb, g]) + locs[(db, g)]
                        s_w = wpool.tile([P, Tg * P], F16, tag="s_t",
                                         name=f"s_{t0}")
                        nc.vector.tensor_tensor(
                            out=s_w[:], in0=iota_h[:, : Tg * P],
                            in1=cr_sb[:, t0 : t0 + Tg].to_broadcast(
                                [P, Tg, P]),
                            op=mybir.AluOpType.is_equal)
                        for t in range(Tg):
                            loc = locs[(db, g)] + t
                            nc.tensor.matmul(
                                out=psum[:],
                                lhsT=gt[:, loc * P : (loc + 1) * P],
                                rhs=s_w[:, t * P : (t + 1) * P],
                                start=False, stop=(k == ntot - 1))
                            k += 1
                    # finalize: fin = aggT @ W.T; h = fin * dinv[dst]
                    aggt = wpool.tile([P, P], F16, tag="aggt",
                                      name=f"aggt_{db}")
                    nc.scalar.activation(aggt[:], psum[:],
                                         mybir.ActivationFunctionType.Copy)
                    fin = ppool.tile([P, P], F32, tag="fin",
                                     name=f"fin_{db}")
                    nc.tensor.matmul(out=fin[:], lhsT=aggt[:], rhs=wt_sb[:],
                                     start=True, stop=True)
                    hb = h_buf[:, db * P : (db + 1) * P]
                    nc.scalar.activation(hb, fin[:],
                                         mybir.ActivationFunctionType.Copy,
                                         scale=dinvb_sb[:, db : db + 1])
                    sq = wpool.tile([P, P], F16, tag="sq",
                                    name=f"sq_{db}")
                    nc.scalar.activation(sq[:], hb,
                                         mybir.ActivationFunctionType.Square)
                    nc.tensor.matmul(out=sh_ps[:], lhsT=ones16[:],
                                     rhs=hb, start=(db == 0),
                                     stop=(db == nblk - 1))
                    nc.tensor.matmul(out=sh2_ps[:], lhsT=ones16[:],
                                     rhs=sq[:], start=(db == 0),
                                     stop=(db == nblk - 1))

            # ---- BN stats reduce + normalize constants
            _no_cc = os.environ.get("KNO_CC", "") == "1"
            sh_sb = cpool.tile([1, D], F32)
            nc.vector.tensor_copy(sh_sb[:], sh_ps[:])
            sh2_sb = cpool.tile([1, D], F32)
            nc.vector.tensor_copy(sh2_sb[:], sh2_ps[:])
            gsum = cpool.tile([1, D], F32)
            gsum2 = cpool.tile([1, D], F32)
            if _no_cc:
                nc.vector.tensor_scalar_mul(gsum[:], sh_sb[:], float(CORES))
                nc.vector.tensor_scalar_mul(gsum2[:], sh2_sb[:], float(CORES))
            else:
                nc.sync.dma_start(out=cc_in[0:1, :], in_=sh_sb[:])
                nc.sync.dma_start(out=cc_in[1:2, :], in_=sh2_sb[:])
                nc.gpsimd.collective_compute(
                    "AllReduce", mybir.AluOpType.add,
                    ins=[cc_in[:]], outs=[cc_out[:]],
                    replica_groups=[list(range(CORES))])
                nc.sync.dma_start(out=gsum[:], in_=cc_out[0:1, :])
                nc.sync.dma_start(out=gsum2[:], in_=cc_out[1:2, :])

            mean = cpool.tile([1, D], F32)
            nc.vector.tensor_scalar_mul(mean[:], gsum[:], 1.0 / N)
            eh2 = cpool.tile([1, D], F32)
            nc.vector.tensor_scalar_mul(eh2[:], gsum2[:], 1.0 / N)
            msq = cpool.tile([1, D], F32)
            nc.vector.tensor_tensor(out=msq[:], in0=mean[:], in1=mean[:],
                                    op=mybir.AluOpType.mult)
            var = cpool.tile([1, D], F32)
            nc.vector.tensor_tensor(out=var[:], in0=eh2[:], in1=msq[:],
                                    op=mybir.AluOpType.subtract)
            vare = cpool.tile([1, D], F32)
            nc.vector.tensor_scalar_add(vare[:], var[:], BN_EPS)
            sdev = cpool.tile([1, D], F32)
            nc.scalar.activation(sdev[:], vare[:],
                                 mybir.ActivationFunctionType.Sqrt)
            rstd = cpool.tile([1, D], F32)
            nc.vector.reciprocal(rstd[:], sdev[:])

            st_row = cpool.tile([1, 2 * D], F32)
            nc.vector.tensor_tensor(out=st_row[:, :D], in0=rstd[:],
                                    in1=gamma_sb[:], op=mybir.AluOpType.mult)
            ms = cpool.tile([1, D], F32)
            nc.vector.tensor_tensor(out=ms[:], in0=mean[:],
                                    in1=st_row[:, :D],
                                    op=mybir.AluOpType.mult)
            nc.vector.tensor_tensor(out=st_row[:, D:], in0=beta_sb[:],
                                    in1=ms[:], op=mybir.AluOpType.subtract)
            bc_ps = ppool.tile([P, 2 * D], F32, tag="fin")
            nc.tensor.matmul(out=bc_ps[:], lhsT=ones_row[:], rhs=st_row[:],
                             start=True, stop=True)
            s_rep = cpool.tile([P, D], F16)
            nc.vector.tensor_copy(s_rep[:], bc_ps[:, :D])
            t_rep = cpool.tile([P, D], F16)
            nc.vector.tensor_copy(t_rep[:], bc_ps[:, D:])
            OW = 1024
            s_w = cpool.tile([P, OW], F16)
            t_w = cpool.tile([P, OW], F16)
            for a in range(OW // P):
                nc.vector.tensor_copy(s_w[:, a * P : (a + 1) * P], s_rep[:])
                nc.vector.tensor_copy(t_w[:, a * P : (a + 1) * P], t_rep[:])

            # ---- phase 3: out = xres + relu(h*s + t), 1024-wide chunks
            nb3 = (npad_local + OW - 1) // OW
            for b in range(nb3):
                w = min(OW, npad_local - b * OW)
                ot = wpool.tile([P, w], F16, tag="ot")
                hslice = h_buf[:, b * OW : b * OW + w]
                tmp = wpool.tile([P, w], F16, tag="p3", name=f"p3_{b}")
                nc.vector.tensor_tensor(out=tmp[:], in0=hslice,
                                        in1=s_w[:, :w],
                                        op=mybir.AluOpType.mult)
                nc.vector.tensor_tensor(out=tmp[:], in0=tmp[:],
                                        in1=t_w[:, :w],
                                        op=mybir.AluOpType.add)
                nc.scalar.activation(ot[:], tmp[:],
                                     mybir.ActivationFunctionType.Relu)
                nc.vector.tensor_tensor(
                    out=ot[:], in0=ot[:],
                    in1=xres_sb[:, b * OW : b * OW + w],
                    op=mybir.AluOpType.add)
                lo = b * OW
                hi = min(npc, lo + w)
                if hi <= lo:
                    continue
                nv = hi - lo
                full = nv // P
                if full > 0:
                    nc.sync.dma_start(
                        out=out_d[lo : lo + full * P, :].rearrange(
                            "(a p) k -> p a k", p=P),
                        in_=ot[:, : full * P].rearrange(
                            "p (a k) -> p a k", k=P))
                rem = nv - full * P
                if rem > 0:
                    nc.sync.dma_start(
                        out=out_d[lo + full * P : hi, :],
                        in_=ot[:rem, full * P : (full + 1) * P])

    nc.compile()
    return nc


# ------------------------------------------------------------------ driver
_CACHE = {}
TRACE = False
RUN_KWARGS = None
LAST_RESULT = None


def kernel(**inputs):
    x = np.asarray(inputs["x"], np.float32)
    edge_index = np.asarray(inputs["edge_index"])
    W = np.asarray(inputs["W"], np.float32)
    gamma = np.asarray(inputs["gamma"], np.float32)
    beta = np.asarray(inputs["beta"], np.float32)
    # inputs["b"] shifts h uniformly and cancels under batch-norm mean
    # subtraction, so it does not affect the output.
    N = x.shape[0]

    plan = _build_plan(x, edge_index, N)
    key = (N, edge_index.shape[1], plan["tiles_total"],
           tuple(plan["T"].ravel().tolist()))
    if key not in _CACHE:
        _CACHE[key] = _build_program(plan)
    nc = _CACHE[key]

    wt16 = np.ascontiguousarray(W.T).astype(np.float16)
    in_maps = []
    for c in range(CORES):
        in_maps.append({
            "xp16": plan["xp16"],
            "xres16": plan["xres16"][c],
            "xo32": plan["xo32"][c],
            "wt16": wt16,
            "gamma": gamma.reshape(1, -1),
            "beta": beta.reshape(1, -1),
            "idx16": plan["idx16"][c],
            "colrel": plan["colrel"][c],
            "dinvb": plan["dinvb"][c],
        })

    res = run_bass_kernel_spmd(nc, in_maps, list(range(CORES)),
                               trace=TRACE, **(RUN_KWARGS or {}))
    global LAST_RESULT
    LAST_RESULT = res
    out = np.concatenate([res.results[c]["out"] for c in range(CORES)],
                         axis=0)
    return out[plan["perm"]].astype(np.float32)
